# revision 2
# baseline (speedup 1.0000x reference)
"""TP-8 Trainium2 Bass kernel for a LLaDA/Llama transformer block.

Shapes (hardcoded): x [2, 1024, 4096], 32 heads x 128 head_dim,
FF=12288, non-causal attention, RMSNorm + RoPE + SwiGLU.

Sharding (per sharding_hint): tensor-parallel over the 8 cores —
q/k/v/ff sharded on the output-feature axis (4 heads / 1536 ff dims per
core), wo/w_out sharded on the contraction axis.  One fp16 on-device
AllReduce PER BATCH restores the residual stream after attention; the
final projection partials are summed on the host.

v2 structure (vs the 2.53ms baseline):
 - norm1 is computed on the host: xnT_h ships pre-normalized, so qkv
   matmuls start immediately and the norm1 ones-matmul pass is gone.
 - The residual stream ships as fp16 (xT_h); output y is fp16 too.
 - One AllReduce per batch, fired right after that batch's o-proj:
   AR(b0) hides under batch-1 qkv/attention, AR(b1) under MLP(b0).
 - norm2 is merged into the MLP block: x_mid tiles are loaded once per
   batch and reused for stats, ff/up matmuls, and the wout residual.
 - Attention is software-pipelined at 512-col chunk granularity
   (lg -> exp -> den/pv with lookahead 3) so the PE doesn't stall on
   the ACT exp latency; v-projection runs first (needs only one xn
   tile to start).
 - Loop orders put the contraction tile outer so each stationary
   operand feeds two consecutive matmuls.
"""

from contextlib import ExitStack

import numpy as np

import concourse.mybir as mybir
import concourse.tile as tile
from concourse import bacc
from concourse.bass_utils import run_bass_kernel_spmd

F32 = mybir.dt.float32
F16 = mybir.dt.float16
AF = mybir.ActivationFunctionType
ALU = mybir.AluOpType

N_CORES = 8
P = 128
B, T, D, FF = 2, 1024, 4096, 12288
M = B * T            # 2048 tokens
H = 128              # head dim
HALF = 64
QC = D // N_CORES    # 512 per-core q/k/v features (4 heads)
NH = QC // H         # 4 heads per core
FC = FF // N_CORES   # 1536 per-core ff features
NKP = D // P         # 32 K-tiles over D
NFT = FC // P        # 12 M-tiles over per-core FF
NDT = D // P         # 32 D-tiles
NST = T // P         # 8 sequence tiles per batch
EPS = 1e-05
LA = 3               # attention pipeline lookahead (512-col chunks)


def _cs(ch):
    return slice(ch * 512, (ch + 1) * 512)


def _build():
    nc = bacc.Bacc("TRN2", target_bir_lowering=False, num_devices=N_CORES)

    xT_h = nc.declare_dram_parameter("xT_h", [D, M], F16, isOutput=False)
    xnT_h = nc.declare_dram_parameter("xnT_h", [D, M], F16, isOutput=False)
    css = nc.declare_dram_parameter("css", [2, P, M], F16, isOutput=False)
    wq_t = nc.declare_dram_parameter("wq_t", [NH, P, NKP, P], F16, isOutput=False)
    wk_t = nc.declare_dram_parameter("wk_t", [NH, P, NKP, P], F16, isOutput=False)
    wv_n = nc.declare_dram_parameter("wv_n", [D, QC], F16, isOutput=False)
    wo_t = nc.declare_dram_parameter("wo_t", [NH, P, NDT, P], F16, isOutput=False)
    wf_t = nc.declare_dram_parameter("wf_t", [NFT, P, NKP, P], F16, isOutput=False)
    wu_t = nc.declare_dram_parameter("wu_t", [NFT, P, NKP, P], F16, isOutput=False)
    wout_t = nc.declare_dram_parameter("wout_t", [NDT, P, NFT, P], F16, isOutput=False)
    y = nc.declare_dram_parameter("y", [D, M], F16, isOutput=True)

    with tile.TileContext(nc) as tc:
        _emit(nc, tc, xT_h, xnT_h, css, wq_t, wk_t, wv_n, wo_t, wf_t, wu_t,
              wout_t, y)
    nc.compile()
    return nc


def _emit(nc, tc, xT_h, xnT_h, css, wq_t, wk_t, wv_n, wo_t, wf_t, wu_t,
          wout_t, y):
    with ExitStack() as top:
        dram_pool = top.enter_context(tc.tile_pool(name="dram", bufs=1, space="DRAM"))
        const = top.enter_context(tc.tile_pool(name="const", bufs=1))

        cc_in = [dram_pool.tile([D, T], F16, name=f"cc_in_{b}") for b in range(B)]
        cc_out = [
            dram_pool.tile([D, T], F16, addr_space="Shared", name=f"cc_out_{b}")
            for b in range(B)
        ]

        ones_h = const.tile([P, P], F16)
        nc.vector.memset(ones_h[:], 1.0)
        cc_sb = const.tile([P, M], F16)
        ss_sb = const.tile([P, M], F16)
        nc.sync.dma_start(out=cc_sb[:], in_=css[0])
        nc.sync.dma_start(out=ss_sb[:], in_=css[1])
        eps_sb = const.tile([P, 1], F32)
        nc.vector.memset(eps_sb[:], EPS)
        bcast2 = [const.tile([P, T], F16, name=f"bcast2_{b}") for b in range(B)]

        # ---------- attention half: qkv + rope + attn + o-proj + AR ----------
        for b in range(B):
            bs = slice(b * T, (b + 1) * T)
            with ExitStack() as bph:
                bp = bph.enter_context(tc.tile_pool(name=f"bat_{b}", bufs=1))
                qf, kf, v_sb = [], [], []
                with ExitStack() as ph:
                    xp = ph.enter_context(tc.tile_pool(name=f"xn_{b}", bufs=1))
                    sp = ph.enter_context(tc.tile_pool(name=f"qkv_{b}", bufs=1))
                    xn = []

                    # v projection first (token-major): starts after one
                    # xn tile + one wv tile have landed.
                    with ExitStack() as vph:
                        vpp = vph.enter_context(
                            tc.tile_pool(name=f"v_ps_{b}", bufs=1, space="PSUM")
                        )
                        ps_v = [
                            vpp.tile([P, QC], F32, tag=f"vps{st}",
                                     name=f"psv_{b}_{st}")
                            for st in range(NST)
                        ]
                        for kp in range(NKP):
                            xnk = xp.tile([P, T], F16, tag=f"xn{kp}",
                                          name=f"xn_{b}_{kp}")
                            nc.sync.dma_start(
                                out=xnk[:], in_=xnT_h[kp * P : (kp + 1) * P, bs]
                            )
                            xn.append(xnk)
                            wvk = sp.tile(
                                [P, QC], F16, tag="wv", bufs=3, name=f"wv_{b}_{kp}"
                            )
                            nc.sync.dma_start(
                                out=wvk[:], in_=wv_n[kp * P : (kp + 1) * P, :]
                            )
                            for st in range(NST):
                                nc.tensor.matmul(
                                    ps_v[st][:],
                                    xn[kp][:, st * P : (st + 1) * P],
                                    wvk[:],
                                    start=(kp == 0),
                                    stop=(kp == NKP - 1),
                                )
                        for st in range(NST):
                            vt = bp.tile(
                                [P, QC], F16, tag=f"v{st}", name=f"v_{b}_{st}"
                            )
                            nc.scalar.copy(vt[:], ps_v[st][:])
                            v_sb.append(vt)

                    # q/k projections, rope fused into the eviction
                    with ExitStack() as qph:
                        qpp = qph.enter_context(
                            tc.tile_pool(name=f"qk_ps_{b}", bufs=1, space="PSUM")
                        )
                        for which, wsrc, dst in (("q", wq_t, qf), ("k", wk_t, kf)):
                            for m in range(NH):
                                wt = sp.tile(
                                    [P, NKP, P], F16, tag="wqk", bufs=3,
                                    name=f"w{which}_{b}_{m}",
                                )
                                nc.sync.dma_start(out=wt[:], in_=wsrc[m])
                                ps = qpp.tile(
                                    [P, T], F32, tag="qk_ps", bufs=2,
                                    name=f"ps{which}_{b}_{m}",
                                )
                                for kp in range(NKP):
                                    for ch in range(T // 512):
                                        nc.tensor.matmul(
                                            ps[:, _cs(ch)],
                                            wt[:, kp, :],
                                            xn[kp][:, _cs(ch)],
                                            start=(kp == 0),
                                            stop=(kp == NKP - 1),
                                        )
                                main = sp.tile(
                                    [P, T], F16, tag="rmain", bufs=2,
                                    name=f"rm_{which}_{b}_{m}",
                                )
                                nc.vector.scalar_tensor_tensor(
                                    main[:], ps[:], 1.0, cc_sb[:, bs],
                                    ALU.mult, ALU.mult,
                                )
                                rot = sp.tile(
                                    [P, T], F16, tag="rrot", bufs=2,
                                    name=f"rr_{which}_{b}_{m}",
                                )
                                nc.vector.scalar_tensor_tensor(
                                    rot[:HALF], ps[HALF:], -1.0,
                                    ss_sb[:HALF, bs], ALU.mult, ALU.mult,
                                )
                                nc.vector.scalar_tensor_tensor(
                                    rot[HALF:], ps[:HALF], 1.0,
                                    ss_sb[HALF:, bs], ALU.mult, ALU.mult,
                                )
                                out = bp.tile(
                                    [P, T], F16, tag=f"{which}f{m}",
                                    name=f"{which}f_{b}_{m}",
                                )
                                nc.vector.tensor_add(out[:], main[:], rot[:])
                                dst.append(out)

                # attention per head, pipelined over 16 512-col chunks
                attnf = []
                afp = bph.enter_context(tc.tile_pool(name=f"attnf_{b}", bufs=1))
                with ExitStack() as ah:
                    ap_ = ah.enter_context(tc.tile_pool(name=f"att_{b}", bufs=1))
                    app = ah.enter_context(
                        tc.tile_pool(name=f"att_ps_{b}", bufs=1, space="PSUM")
                    )
                    for h in range(NH):
                        den = [
                            app.tile([P, 512], F32, tag=f"den{ch}",
                                     name=f"den_{b}_{h}_{ch}")
                            for ch in range(2)
                        ]
                        at = [
                            app.tile([P, 512], F32, tag=f"at{ch}",
                                     name=f"at_{b}_{h}_{ch}")
                            for ch in range(2)
                        ]

                        def emit_lg(j, b=b, h=h):
                            st, ch = divmod(j, 2)
                            lg = app.tile(
                                [P, 512], F32, tag="lg", bufs=4,
                                name=f"lg_{b}_{h}_{j}",
                            )
                            nc.tensor.matmul(
                                lg[:],
                                kf[h][:, st * P : (st + 1) * P],
                                qf[h][:, _cs(ch)],
                                start=True,
                                stop=True,
                            )
                            pr = ap_.tile(
                                [P, 512], F16, tag="pr", bufs=6,
                                name=f"pr_{b}_{h}_{j}",
                            )
                            nc.scalar.activation(pr[:], lg[:], AF.Exp)
                            return pr

                        prs = [None] * 16
                        for j in range(LA):
                            prs[j] = emit_lg(j)
                        for j in range(16):
                            if j + LA < 16:
                                prs[j + LA] = emit_lg(j + LA)
                            st, ch = divmod(j, 2)
                            pr = prs[j]
                            nc.tensor.matmul(
                                den[ch][:], ones_h[:], pr[:],
                                start=(st == 0), stop=(st == NST - 1),
                            )
                            nc.tensor.matmul(
                                at[ch][:],
                                v_sb[st][:, h * H : (h + 1) * H],
                                pr[:],
                                start=(st == 0), stop=(st == NST - 1),
                            )
                            prs[j] = None
                        af = afp.tile([P, T], F16, tag=f"af{h}", name=f"af_{b}_{h}")
                        for ch in range(2):
                            rec = ap_.tile(
                                [P, 512], F32, tag="rec", bufs=4,
                                name=f"rec_{b}_{h}_{ch}",
                            )
                            nc.vector.reciprocal(rec[:], den[ch][:])
                            nc.vector.scalar_tensor_tensor(
                                af[:, _cs(ch)], at[ch][:], 1.0, rec[:],
                                ALU.mult, ALU.mult,
                            )
                        attnf.append(af)

                # o-projection partial + residual, then the batch AllReduce
                with ExitStack() as ph:
                    sp = ph.enter_context(tc.tile_pool(name=f"op_{b}", bufs=1))
                    pp = ph.enter_context(
                        tc.tile_pool(name=f"op_ps_{b}", bufs=1, space="PSUM")
                    )
                    wo_sb = []
                    for h in range(NH):
                        wt = sp.tile(
                            [P, NDT, P], F16, tag=f"wo{h}", name=f"wo_{b}_{h}"
                        )
                        nc.sync.dma_start(out=wt[:], in_=wo_t[h])
                        wo_sb.append(wt)
                    for dt in range(NDT):
                        ps = pp.tile(
                            [P, T], F32, tag="o_ps", bufs=2, name=f"pso_{b}_{dt}"
                        )
                        for h in range(NH):
                            for ch in range(T // 512):
                                nc.tensor.matmul(
                                    ps[:, _cs(ch)],
                                    wo_sb[h][:, dt, :],
                                    attnf[h][:, _cs(ch)],
                                    start=(h == 0),
                                    stop=(h == NH - 1),
                                )
                        xt = sp.tile(
                            [P, T], F16, tag="xs3", bufs=3, name=f"xo_{b}_{dt}"
                        )
                        nc.sync.dma_start(
                            out=xt[:], in_=xT_h[dt * P : (dt + 1) * P, bs]
                        )
                        osb = sp.tile(
                            [P, T], F16, tag="osb", bufs=3, name=f"osb_{b}_{dt}"
                        )
                        nc.vector.scalar_tensor_tensor(
                            osb[:], xt[:], 1.0 / N_CORES, ps[:],
                            ALU.mult, ALU.add,
                        )
                        nc.sync.dma_start(
                            out=cc_in[b][dt * P : (dt + 1) * P, :], in_=osb[:]
                        )
                    nc.gpsimd.collective_compute(
                        "AllReduce",
                        ALU.add,
                        replica_groups=[list(range(N_CORES))],
                        ins=[cc_in[b][:, :]],
                        outs=[cc_out[b][:, :]],
                    )

        # ---------------- norm2 + SwiGLU MLP, per batch ----------------
        for b in range(B):
            bs = slice(b * T, (b + 1) * T)
            with ExitStack() as bph:
                bp = bph.enter_context(tc.tile_pool(name=f"mlpb_{b}", bufs=1))
                stp = bph.enter_context(tc.tile_pool(name=f"mstat_{b}", bufs=1))
                spp = bph.enter_context(
                    tc.tile_pool(name=f"mstat_ps_{b}", bufs=1, space="PSUM")
                )
                # x_mid tiles: loaded once, reused by stats, ff/up matmuls
                # and the wout residual.
                xmh = []
                ms_ps = spp.tile([P, T], F32, name=f"ms_{b}")
                for kp in range(NKP):
                    xk = bp.tile([P, T], F16, tag=f"xm{kp}", name=f"xmh_{b}_{kp}")
                    nc.sync.dma_start(
                        out=xk[:], in_=cc_out[b][kp * P : (kp + 1) * P, :]
                    )
                    xmh.append(xk)
                    sq = stp.tile([P, T], F16, tag="sq", bufs=3,
                                  name=f"sq_{b}_{kp}")
                    if kp % 2 == 0:
                        nc.scalar.activation(sq[:], xk[:], AF.Square)
                    else:
                        nc.vector.tensor_mul(sq[:], xk[:], xk[:])
                    for ch in range(2):
                        nc.tensor.matmul(
                            ms_ps[:, _cs(ch)], ones_h[:], sq[:, _cs(ch)],
                            start=(kp == 0), stop=(kp == NKP - 1),
                        )
                lnt = stp.tile([P, T], F32, name=f"lnt_{b}")
                nc.scalar.activation(
                    lnt[:], ms_ps[:], AF.Ln, bias=eps_sb[:], scale=1.0 / D
                )
                nc.scalar.activation(bcast2[b][:], lnt[:], AF.Exp, scale=-0.5)

                hsb = []
                with ExitStack() as ph:
                    sp = ph.enter_context(tc.tile_pool(name=f"mlp_{b}", bufs=1))
                    pp = ph.enter_context(
                        tc.tile_pool(name=f"mlp_ps_{b}", bufs=1, space="PSUM")
                    )
                    ffs = []
                    for m in range(NFT):
                        for which, wsrc in (("f", wf_t), ("u", wu_t)):
                            wt = sp.tile(
                                [P, NKP, P], F16, tag="wffu", bufs=3,
                                name=f"w{which}_{b}_{m}",
                            )
                            nc.sync.dma_start(out=wt[:], in_=wsrc[m])
                            ps = pp.tile(
                                [P, T], F32, tag="ps_fu", bufs=2,
                                name=f"ps{which}_{b}_{m}",
                            )
                            for kp in range(NKP):
                                for ch in range(2):
                                    nc.tensor.matmul(
                                        ps[:, _cs(ch)],
                                        wt[:, kp, :],
                                        xmh[kp][:, _cs(ch)],
                                        start=(kp == 0),
                                        stop=(kp == NKP - 1),
                                    )
                            # fold the norm2 scale into the eviction
                            nt = sp.tile(
                                [P, T], F16, tag=f"nrm_{which}", bufs=3,
                                name=f"nt{which}_{b}_{m}",
                            )
                            nc.vector.scalar_tensor_tensor(
                                nt[:], ps[:], 1.0, bcast2[b][:],
                                ALU.mult, ALU.mult,
                            )
                            if which == "f":
                                ft = sp.tile(
                                    [P, T], F16, tag="ffs", bufs=3,
                                    name=f"ff_{b}_{m}",
                                )
                                nc.scalar.activation(ft[:], nt[:], AF.Silu)
                                ffs.append(ft)
                            else:
                                ht = bp.tile(
                                    [P, T], F16, tag=f"h{m}", name=f"h_{b}_{m}"
                                )
                                nc.vector.tensor_mul(ht[:], nt[:], ffs[m][:])
                                hsb.append(ht)

                # w_out projection + residual, partial fp16 output
                with ExitStack() as ph:
                    sp = ph.enter_context(tc.tile_pool(name=f"wo2_{b}", bufs=1))
                    pp = ph.enter_context(
                        tc.tile_pool(name=f"wo2_ps_{b}", bufs=1, space="PSUM")
                    )
                    for dt in range(NDT):
                        wt = sp.tile(
                            [P, NFT, P], F16, tag="wot", bufs=3,
                            name=f"wot_{b}_{dt}",
                        )
                        nc.sync.dma_start(out=wt[:], in_=wout_t[dt])
                        ps = pp.tile(
                            [P, T], F32, tag="ps_o2", bufs=2, name=f"pso2_{b}_{dt}"
                        )
                        for m in range(NFT):
                            for ch in range(2):
                                nc.tensor.matmul(
                                    ps[:, _cs(ch)],
                                    wt[:, m, :],
                                    hsb[m][:, _cs(ch)],
                                    start=(m == 0),
                                    stop=(m == NFT - 1),
                                )
                        ysb = sp.tile(
                            [P, T], F16, tag="ysb", bufs=3, name=f"ysb_{b}_{dt}"
                        )
                        nc.vector.scalar_tensor_tensor(
                            ysb[:], xmh[dt][:], 1.0 / N_CORES, ps[:],
                            ALU.mult, ALU.add,
                        )
                        nc.sync.dma_start(
                            out=y[dt * P : (dt + 1) * P, bs], in_=ysb[:]
                        )


_NC_CACHE = {}


def _get_nc():
    if "nc" not in _NC_CACHE:
        _NC_CACHE["nc"] = _build()
    return _NC_CACHE["nc"]


def _host_prep(x, sin, cos, attn_norm_w, ff_norm_w, wq, wk, wv, wo, w_ff, w_up, w_out):
    f16 = np.float16
    x2 = np.asarray(x, np.float32).reshape(M, D)
    xT = np.ascontiguousarray(x2.T)
    rs1 = 1.0 / np.sqrt((xT * xT).mean(0) + EPS)        # [M] norm1 on host
    xnT = xT * rs1[None, :]

    sinT = np.asarray(sin, np.float32).reshape(M, HALF).T
    cosT = np.asarray(cos, np.float32).reshape(M, HALF).T
    cc = np.concatenate([cosT, cosT], axis=0)
    ss = np.concatenate([sinT, sinT], axis=0)
    css = np.stack([cc, ss]).astype(f16)

    anw = np.asarray(attn_norm_w, np.float32)[:, None]
    fnw = np.asarray(ff_norm_w, np.float32)[:, None]
    wqn = (anw * np.asarray(wq, np.float32)) * (H ** -0.5)
    wkn = anw * np.asarray(wk, np.float32)
    wvn = anw * np.asarray(wv, np.float32)
    wfn = fnw * np.asarray(w_ff, np.float32)
    wun = fnw * np.asarray(w_up, np.float32)
    wo = np.asarray(wo, np.float32)
    w_out = np.asarray(w_out, np.float32)

    def mtile(w):
        # [K, F] -> [F/P, P, K/P, P] with [m, p, kp, j] = w[kp*P+p, m*P+j]
        K, F = w.shape
        return np.ascontiguousarray(
            w.reshape(K // P, P, F // P, P).transpose(2, 1, 0, 3)
        )

    in_maps = []
    for c in range(N_CORES):
        qs = slice(c * QC, (c + 1) * QC)
        fs = slice(c * FC, (c + 1) * FC)
        in_maps.append(
            {
                "xT_h": xT.astype(f16),
                "xnT_h": xnT.astype(f16),
                "css": css,
                "wq_t": mtile(wqn[:, qs]).astype(f16),
                "wk_t": mtile(wkn[:, qs]).astype(f16),
                "wv_n": wvn[:, qs].astype(f16),
                # [h, p, dt, j] = wo[c*QC + h*P + p, dt*P + j]
                "wo_t": np.ascontiguousarray(
                    wo[qs, :].reshape(NH, P, NDT, P)
                ).astype(f16),
                "wf_t": mtile(wfn[:, fs]).astype(f16),
                "wu_t": mtile(wun[:, fs]).astype(f16),
                "wout_t": mtile(w_out[fs, :]).astype(f16),
            }
        )
    return in_maps


def kernel(**inputs) -> np.ndarray:
    nc = _get_nc()
    in_maps = _host_prep(**inputs)
    res = run_bass_kernel_spmd(
        nc, in_maps, core_ids=list(range(N_CORES)), trace=False
    )
    acc = res.results[0]["y"].astype(np.float64)
    for c in range(1, N_CORES):
        acc += res.results[c]["y"]
    return np.ascontiguousarray(acc.T).astype(np.float32).reshape(B, T, D)
